# revision 5
# baseline (speedup 1.0000x reference)
"""AttentionGate fused kernel for Trainium2, data-parallel over batch on 8 NeuronCores.

Reference computation (per pixel p, channels c=0..255):
    att   = relu(Wx@x + Wg@g + bxg)            # [C] per pixel
    attn  = LN_c(att) * gamma + beta           # layernorm over channels
    psi   = sigmoid(Wpsi @ attn + bpsi)        # scalar per pixel
    out   = psi * x

Key algebraic fold: attn never needs materializing. With
    Wpg  = Wpsi * gamma
    s2   = sum(Wpg)
    colE = Wpg - s2/C
    s1   = Wpsi @ beta + bpsi
we have   psi = sigmoid( rstd * (colE @ att) + s1 ),
          rstd = rsqrt(mean(att^2) - mean(att)^2 + eps)
so per pixel we only need three channel reductions of att / att^2, which are
computed on the TensorEngine as matmuls with skinny stat vectors.

Layout: channels on partitions, pixels on the free dim (natural NCHW layout,
no transposes anywhere). Main 1x1-conv GEMMs run in float32r (tf32-like,
1 cycle/row). Stats + psi-broadcast run in bf16.
"""

import os
import sys
import types

import numpy as np
import ml_dtypes

import concourse.bass as bass
import concourse.tile as tile
from concourse import mybir, bacc
from concourse.bass_utils import run_bass_kernel_spmd

F32 = mybir.dt.float32
F32R = mybir.dt.float32r
BF16 = mybir.dt.bfloat16

N_CORES = 8
B, C, H, W = 16, 256, 64, 64
LN_EPS = 1e-5
BS = B // N_CORES          # batches per core = 2
PIX = H * W                # 4096 pixels per batch plane
NTILE_PER_B = PIX // 512   # 8 pixel tiles per batch
NT = BS * NTILE_PER_B      # 16 pixel tiles per core
NPT = 512                  # pixels per tile


def _register_ntff_hook():
    """Register the axon NTFF profile hook if the image's antenv lacks it."""
    try:
        from antenv.axon_hooks import get_axon_ntff_profile_hook  # noqa: F401
        return
    except ImportError:
        pass
    try:
        import antenv  # noqa: F401
        mod = types.ModuleType("antenv.axon_hooks")
        _h = [None]
        mod.set_axon_ntff_profile_hook = lambda h: _h.__setitem__(0, h)
        mod.get_axon_ntff_profile_hook = lambda: _h[0]
        sys.modules["antenv.axon_hooks"] = mod
        from trn_agent_boot.trn_boot import _ntff_profile_via_ctypes
        mod.set_axon_ntff_profile_hook(
            _ntff_profile_via_ctypes("/opt/axon/libaxon_pjrt.so")
        )
    except Exception:
        pass


def _build():
    nc = bacc.Bacc("TRN2", target_bir_lowering=False, debug=False,
                   num_devices=N_CORES)

    x_d = nc.dram_tensor("x", [BS, C, PIX], F32R, kind="ExternalInput").ap()
    g_d = nc.dram_tensor("g", [BS, C, PIX], F32R, kind="ExternalInput").ap()
    # gemm weights: [src(2: x,g), j(2: out-ch block), k(2: in-ch block), 128c, 128o]
    gw_d = nc.dram_tensor("gw", [2, 2, 2, 128, 128], F32R, kind="ExternalInput").ap()
    # stat lhsT, zero-padded so tile t's stats land on psum partition t:
    # sm[t, c, m] = (m==t)/C            (used for mean and for meansq)
    # se[j, t, c, m] = (m==t)*colE[128j + c]
    sm_d = nc.dram_tensor("sm", [NT, 128, NT], BF16, kind="ExternalInput").ap()
    se_d = nc.dram_tensor("se", [2, NT, 128, NT], BF16, kind="ExternalInput").ap()
    # indicator broadcast lhsT: ind[t, k, m] = (k==t)
    ind_d = nc.dram_tensor("ind", [NT, NT, 128], BF16, kind="ExternalInput").ap()
    bias_d = nc.dram_tensor("bxg2", [2, 128], F32, kind="ExternalInput").ap()
    scal_d = nc.dram_tensor("scal", [2], F32, kind="ExternalInput").ap()  # [s1, eps]
    out_d = nc.dram_tensor("out", [BS, C, PIX], F32, kind="ExternalOutput").ap()

    from contextlib import ExitStack
    with tile.TileContext(nc) as tc, ExitStack() as ctx:
        consts = ctx.enter_context(tc.tile_pool(name="consts", bufs=1))
        xpool = ctx.enter_context(tc.tile_pool(name="xres", bufs=1))
        gpool = ctx.enter_context(tc.tile_pool(name="g", bufs=4))
        attp = ctx.enter_context(tc.tile_pool(name="att", bufs=4))
        sqp = ctx.enter_context(tc.tile_pool(name="attsq", bufs=4))
        outp = ctx.enter_context(tc.tile_pool(name="o", bufs=4))
        psip = ctx.enter_context(tc.tile_pool(name="psi", bufs=1))
        bigps = ctx.enter_context(tc.tile_pool(name="bigps", bufs=3, space="PSUM"))
        bcastps = ctx.enter_context(tc.tile_pool(name="bcastps", bufs=2, space="PSUM"))
        statps = ctx.enter_context(tc.tile_pool(name="statps", bufs=1, space="PSUM"))

        # ---- constants to SBUF ----
        gwt = {}
        for s in range(2):
            for j in range(2):
                for k in range(2):
                    t_ = consts.tile([128, 128], F32R, tag=f"gw{s}{j}{k}")
                    nc.sync.dma_start(t_[:], gw_d[s, j, k])
                    gwt[(s, j, k)] = t_
        smt = []
        for t in range(NT):
            t_ = consts.tile([128, NT], BF16, tag=f"sm{t}")
            nc.sync.dma_start(t_[:], sm_d[t])
            smt.append(t_)
        sett = {}
        for j in range(2):
            for t in range(NT):
                t_ = consts.tile([128, NT], BF16, tag=f"se{j}_{t}")
                nc.sync.dma_start(t_[:], se_d[j, t])
                sett[(j, t)] = t_
        indt = []
        for t in range(NT):
            t_ = consts.tile([NT, 128], BF16, tag=f"ind{t}")
            nc.sync.dma_start(t_[:], ind_d[t])
            indt.append(t_)
        biast = []
        for j in range(2):
            t_ = consts.tile([128, 1], F32, tag=f"bias{j}")
            nc.sync.dma_start(t_[:], bias_d[j].unsqueeze(1))
            biast.append(t_)
        s1t = consts.tile([NT, 1], F32, tag="s1")
        nc.sync.dma_start(s1t[:], scal_d[0:1].unsqueeze(0).partition_broadcast(NT))
        epst = consts.tile([NT, 1], F32, tag="eps")
        nc.sync.dma_start(epst[:], scal_d[1:2].unsqueeze(0).partition_broadcast(NT))

        # dummy sqrt to preload the sqrt_and_others ACT table set (has relu too),
        # so phase-1 relus don't trigger a different set and the real Sqrt is free
        dummy = consts.tile([1, 1], F32, tag="dummy")
        nc.vector.memset(dummy[:], 1.0)
        nc.scalar.activation(dummy[:], dummy[:], mybir.ActivationFunctionType.Sqrt)

        # x stays resident in SBUF for the output phase: layout
        # [128, (b*2+k)*PIX + pix]
        xres = xpool.tile([128, BS * 2 * PIX], F32R, tag="xres")

        ps_mean = statps.tile([NT, NPT], F32, tag="mean")
        ps_e = statps.tile([NT, NPT], F32, tag="e")
        ps_q = statps.tile([NT, NPT], F32, tag="q")

        # ---- phase 1: GEMMs + relu + stats ----
        for t in range(NT):
            b, p0 = t // NTILE_PER_B, (t % NTILE_PER_B) * NPT
            xsl = []
            for k in range(2):
                sl = xres[:, (b * 2 + k) * PIX + p0:(b * 2 + k) * PIX + p0 + NPT]
                nc.sync.dma_start(sl, x_d[b, 128 * k:128 * (k + 1), p0:p0 + NPT])
                xsl.append(sl)
            gt = []
            for k in range(2):
                t_ = gpool.tile([128, NPT], F32R, tag=f"g{k}")
                nc.sync.dma_start(t_[:], g_d[b, 128 * k:128 * (k + 1), p0:p0 + NPT])
                gt.append(t_)
            first = t == 0
            last = t == NT - 1
            for j in range(2):
                ps = bigps.tile([128, NPT], F32, tag="gemm")
                nc.tensor.matmul(ps[:], gwt[(0, j, 0)][:], xsl[0], start=True, stop=False)
                nc.tensor.matmul(ps[:], gwt[(0, j, 1)][:], xsl[1], start=False, stop=False)
                nc.tensor.matmul(ps[:], gwt[(1, j, 0)][:], gt[0][:], start=False, stop=False)
                nc.tensor.matmul(ps[:], gwt[(1, j, 1)][:], gt[1][:], start=False, stop=True)
                att = attp.tile([128, NPT], BF16, tag=f"att{j}")
                nc.scalar.activation(att[:], ps[:], mybir.ActivationFunctionType.Relu,
                                     bias=biast[j][:], scale=1.0)
                attsq = sqp.tile([128, NPT], BF16, tag=f"sq{j}")
                nc.vector.tensor_mul(attsq[:], att[:], att[:])
                fj = first and j == 0
                lj = last and j == 1
                nc.tensor.matmul(ps_mean[:], smt[t][:], att[:],
                                 start=fj, stop=lj, skip_group_check=True)
                nc.tensor.matmul(ps_e[:], sett[(j, t)][:], att[:],
                                 start=fj, stop=lj, skip_group_check=True)
                nc.tensor.matmul(ps_q[:], smt[t][:], attsq[:],
                                 start=fj, stop=lj, skip_group_check=True)

        # ---- phase 2: psi for all 16*512 pixel-tiles at once ----
        p1 = psip.tile([NT, NPT], F32, tag="p1")
        nc.scalar.activation(p1[:], ps_mean[:], mybir.ActivationFunctionType.Square)
        v = psip.tile([NT, NPT], F32, tag="v")
        nc.vector.tensor_tensor(v[:], ps_q[:], p1[:], op=mybir.AluOpType.subtract)
        sd = psip.tile([NT, NPT], F32, tag="sd")
        nc.scalar.activation(sd[:], v[:], mybir.ActivationFunctionType.Sqrt,
                             bias=epst[:], scale=1.0)
        r = psip.tile([NT, NPT], F32, tag="r")
        nc.vector.reciprocal(r[:], sd[:])
        z = psip.tile([NT, NPT], F32, tag="z")
        nc.vector.tensor_mul(z[:], ps_e[:], r[:])
        psi = psip.tile([NT, NPT], BF16, tag="psi")
        nc.scalar.activation(psi[:], z[:], mybir.ActivationFunctionType.Sigmoid,
                             bias=s1t[:], scale=1.0)

        # ---- phase 3: broadcast psi, gate x, store ----
        for t in range(NT):
            b, p0 = t // NTILE_PER_B, (t % NTILE_PER_B) * NPT
            bc = bcastps.tile([128, NPT], F32, tag="bcast")
            nc.tensor.matmul(bc[:], indt[t][:], psi[:], start=True, stop=True,
                             skip_group_check=True)
            for k in range(2):
                xs = xres[:, (b * 2 + k) * PIX + p0:(b * 2 + k) * PIX + p0 + NPT]
                ot = outp.tile([128, NPT], F32, tag=f"o{k}")
                nc.vector.tensor_mul(ot[:], xs.bitcast(F32), bc[:])
                nc.sync.dma_start(out_d[b, 128 * k:128 * (k + 1), p0:p0 + NPT], ot[:])

    nc.compile()
    return nc


_NC = None
LAST_RESULT = None


def kernel(x, g, Wx, Wg, Wpsi, ln_gamma, ln_beta, bxg, bpsi):
    global _NC, LAST_RESULT
    _register_ntff_hook()
    if _NC is None:
        _NC = _build()

    x = np.ascontiguousarray(np.asarray(x, dtype=np.float32))
    g = np.ascontiguousarray(np.asarray(g, dtype=np.float32))
    Wx = np.asarray(Wx, dtype=np.float32)
    Wg = np.asarray(Wg, dtype=np.float32)
    Wpsi = np.asarray(Wpsi, dtype=np.float32)
    ln_gamma = np.asarray(ln_gamma, dtype=np.float32)
    ln_beta = np.asarray(ln_beta, dtype=np.float32)
    bxg = np.asarray(bxg, dtype=np.float32)
    bpsi = np.asarray(bpsi, dtype=np.float32)

    # host-side folds
    Wpg = Wpsi[0] * ln_gamma                      # [C]
    s2 = float(Wpg.sum())
    colE = (Wpg - s2 / C).astype(np.float32)      # [C]
    s1 = float(Wpsi[0] @ ln_beta + bpsi[0])

    gw = np.empty((2, 2, 2, 128, 128), np.float32)
    for s, Wsrc in enumerate((Wx, Wg)):
        for j in range(2):
            for k in range(2):
                gw[s, j, k] = Wsrc[128 * j:128 * (j + 1), 128 * k:128 * (k + 1)].T
    sm = np.zeros((NT, 128, NT), np.float32)
    se = np.zeros((2, NT, 128, NT), np.float32)
    for t in range(NT):
        sm[t, :, t] = 1.0 / C
        for j in range(2):
            se[j, t, :, t] = colE[128 * j:128 * (j + 1)]
    ind = np.zeros((NT, NT, 128), np.float32)
    for t in range(NT):
        ind[t, t, :] = 1.0
    bias2 = np.stack([bxg[:128], bxg[128:]])      # [2, 128]
    scal = np.array([s1, LN_EPS], np.float32)

    bf = ml_dtypes.bfloat16
    xr = x.reshape(B, C, PIX)
    gr = g.reshape(B, C, PIX)
    in_maps = []
    for i in range(N_CORES):
        in_maps.append({
            "x": xr[i * BS:(i + 1) * BS],
            "g": gr[i * BS:(i + 1) * BS],
            "gw": gw,
            "sm": sm.astype(bf),
            "se": se.astype(bf),
            "ind": ind.astype(bf),
            "bxg2": bias2,
            "scal": scal,
        })

    trace = bool(os.environ.get("KERNEL_TRACE"))
    res = run_bass_kernel_spmd(_NC, in_maps, list(range(N_CORES)), trace=trace)
    LAST_RESULT = res
    out = np.concatenate([np.asarray(res.results[i]["out"]) for i in range(N_CORES)],
                         axis=0)
    return out.reshape(B, C, H, W)


# revision 7
# speedup vs baseline: 1.3222x; 1.3222x over previous
"""AttentionGate fused kernel for Trainium2, data-parallel over batch on 8 NeuronCores.

Reference computation (per pixel p, channels c=0..255):
    att   = relu(Wx@x + Wg@g + bxg)            # [C] per pixel
    attn  = LN_c(att) * gamma + beta           # layernorm over channels
    psi   = sigmoid(Wpsi @ attn + bpsi)        # scalar per pixel
    out   = psi * x

Key algebraic fold: attn never needs materializing. With
    Wpg  = Wpsi * gamma
    s2   = sum(Wpg)
    colE = Wpg - s2/C
    s1   = Wpsi @ beta + bpsi
we have   psi = sigmoid( rstd * (colE @ att) + s1 ),
          rstd = rsqrt(mean(att^2) - mean(att)^2 + eps)
so per pixel we only need three channel reductions of att / att^2, computed on
the TensorEngine as matmuls with skinny (zero-padded) stat vectors so tile t's
stats land on psum partition t.

Layout: channels on partitions, pixels on the free dim (natural NCHW layout,
no transposes anywhere). Main 1x1-conv GEMMs in float32r (tf32-like,
1 cycle/row); stats + psi-broadcast matmuls in bf16.
"""

import os
import sys
import types
from contextlib import ExitStack

import numpy as np
import ml_dtypes

import concourse.bass as bass
import concourse.tile as tile
from concourse import mybir, bacc
from concourse.bass_utils import run_bass_kernel_spmd

F32 = mybir.dt.float32
F32R = mybir.dt.float32r
BF16 = mybir.dt.bfloat16

N_CORES = 8
B, C, H, W = 16, 256, 64, 64
LN_EPS = 1e-5
BS = B // N_CORES          # batches per core = 2
PIX = H * W                # 4096 pixels per batch plane
NPT = 512                  # pixels per tile
NTILE_PER_B = PIX // NPT   # 8 pixel tiles per batch
NT = BS * NTILE_PER_B      # 16 pixel tiles per core
CHUNK = 2048               # pixels per big DMA chunk (1 MB f32)
TPC = CHUNK // NPT         # tiles per chunk = 4


def _register_ntff_hook():
    """Register the axon NTFF profile hook if the image's antenv lacks it."""
    try:
        from antenv.axon_hooks import get_axon_ntff_profile_hook  # noqa: F401
        return
    except ImportError:
        pass
    try:
        import antenv  # noqa: F401
        mod = types.ModuleType("antenv.axon_hooks")
        _h = [None]
        mod.set_axon_ntff_profile_hook = lambda h: _h.__setitem__(0, h)
        mod.get_axon_ntff_profile_hook = lambda: _h[0]
        sys.modules["antenv.axon_hooks"] = mod
        from trn_agent_boot.trn_boot import _ntff_profile_via_ctypes
        mod.set_axon_ntff_profile_hook(
            _ntff_profile_via_ctypes("/opt/axon/libaxon_pjrt.so")
        )
    except Exception:
        pass


def _build():
    nc = bacc.Bacc("TRN2", target_bir_lowering=False, debug=False,
                   num_devices=N_CORES)

    x_d = nc.dram_tensor("x", [BS, C, PIX], F32R, kind="ExternalInput").ap()
    g_d = nc.dram_tensor("g", [BS, C, PIX], F32R, kind="ExternalInput").ap()
    # packed constants (see kernel() for the host-side layout):
    # gw: [128c, (s*2+j)*2+k slabs of 128o]   (weights, transposed per-slab)
    gw_d = nc.dram_tensor("gw", [128, 8 * 128], F32R, kind="ExternalInput").ap()
    # sm[c, t*16+m] = (m==t)/C ; se[c, (j*16+t)*16+m] = (m==t)*colE[128j+c]
    sm_d = nc.dram_tensor("sm", [128, NT * NT], BF16, kind="ExternalInput").ap()
    se_d = nc.dram_tensor("se", [128, 2 * NT * NT], BF16, kind="ExternalInput").ap()
    # ind[k, t*128+m] = (k==t)
    ind_d = nc.dram_tensor("ind", [NT, NT * 128], BF16, kind="ExternalInput").ap()
    bias_d = nc.dram_tensor("bxg2", [2, 128], F32, kind="ExternalInput").ap()
    scal_d = nc.dram_tensor("scal", [2], F32, kind="ExternalInput").ap()  # [s1, eps]
    out_d = nc.dram_tensor("out", [BS, C, PIX], F32, kind="ExternalOutput").ap()

    with tile.TileContext(nc) as tc, ExitStack() as ctx:
        consts = ctx.enter_context(tc.tile_pool(name="consts", bufs=1))
        xpool = ctx.enter_context(tc.tile_pool(name="xres", bufs=1))
        gpool = ctx.enter_context(tc.tile_pool(name="g", bufs=2))
        attp = ctx.enter_context(tc.tile_pool(name="att", bufs=3))
        sqp = ctx.enter_context(tc.tile_pool(name="attsq", bufs=3))
        outp = ctx.enter_context(tc.tile_pool(name="o", bufs=1))
        psip = ctx.enter_context(tc.tile_pool(name="psi", bufs=1))
        bigps = ctx.enter_context(tc.tile_pool(name="bigps", bufs=3, space="PSUM"))
        bcastps = ctx.enter_context(tc.tile_pool(name="bcastps", bufs=2, space="PSUM"))
        statps = ctx.enter_context(tc.tile_pool(name="statps", bufs=1, space="PSUM"))

        # ---- constants to SBUF (one coalesced DMA each) ----
        gw_all = consts.tile([128, 8 * 128], F32R, tag="gw")
        nc.sync.dma_start(gw_all[:], gw_d[:])
        sm_all = consts.tile([128, NT * NT], BF16, tag="sm")
        nc.sync.dma_start(sm_all[:], sm_d[:])
        se_all = consts.tile([128, 2 * NT * NT], BF16, tag="se")
        nc.sync.dma_start(se_all[:], se_d[:])
        ind_all = consts.tile([NT, NT * 128], BF16, tag="ind")
        nc.sync.dma_start(ind_all[:], ind_d[:])

        def gwt(s, j, k):
            i = (s * 2 + j) * 2 + k
            return gw_all[:, i * 128:(i + 1) * 128]

        def smt(t):
            return sm_all[:, t * NT:(t + 1) * NT]

        def sett(j, t):
            i = j * NT + t
            return se_all[:, i * NT:(i + 1) * NT]

        def indt(t):
            return ind_all[:, t * 128:(t + 1) * 128]

        biast = []
        for j in range(2):
            t_ = consts.tile([128, 1], F32, tag=f"bias{j}")
            nc.sync.dma_start(t_[:], bias_d[j].unsqueeze(1))
            biast.append(t_)
        s1t = consts.tile([NT, 1], F32, tag="s1")
        nc.sync.dma_start(s1t[:], scal_d[0:1].unsqueeze(0).partition_broadcast(NT))
        epst = consts.tile([NT, 1], F32, tag="eps")
        nc.sync.dma_start(epst[:], scal_d[1:2].unsqueeze(0).partition_broadcast(NT))

        # dummy sqrt to preload the sqrt_and_others ACT table set (has relu too),
        # so phase-1 relus don't load a different set and the real Sqrt is free
        dummy = consts.tile([1, 1], F32, tag="dummy")
        nc.vector.memset(dummy[:], 1.0)
        nc.scalar.activation(dummy[:], dummy[:], mybir.ActivationFunctionType.Sqrt)

        # x and out stay resident in SBUF; free-dim layout [(b*2+k)*PIX + pix]
        xres = xpool.tile([128, BS * 2 * PIX], F32R, tag="xres")
        outres = outp.tile([128, BS * 2 * PIX], F32, tag="outres")

        ps_mean = statps.tile([NT, NPT], F32, tag="mean")
        ps_e = statps.tile([NT, NPT], F32, tag="e")
        ps_q = statps.tile([NT, NPT], F32, tag="q")

        def off(b, k, p0):
            return (b * 2 + k) * PIX + p0

        # ---- input loads: 1 MB chunks, interleaved so early tiles unblock fast
        gch = {}
        for b in range(BS):
            for h in range(PIX // CHUNK):
                p0 = h * CHUNK
                for k in range(2):
                    nc.sync.dma_start(
                        xres[:, off(b, k, p0):off(b, k, p0) + CHUNK],
                        x_d[b, 128 * k:128 * (k + 1), p0:p0 + CHUNK])
                for k in range(2):
                    t_ = gpool.tile([128, CHUNK], F32R, tag=f"g{k}")
                    nc.sync.dma_start(
                        t_[:], g_d[b, 128 * k:128 * (k + 1), p0:p0 + CHUNK])
                    gch[(b, h, k)] = t_

        # ---- phase 1: GEMMs + relu + stats ----
        for t in range(NT):
            b, p0 = t // NTILE_PER_B, (t % NTILE_PER_B) * NPT
            h, q0 = p0 // CHUNK, p0 % CHUNK
            xsl = [xres[:, off(b, k, p0):off(b, k, p0) + NPT] for k in range(2)]
            gsl = [gch[(b, h, k)][:, q0:q0 + NPT] for k in range(2)]
            first, last = t == 0, t == NT - 1
            for j in range(2):
                ps = bigps.tile([128, NPT], F32, tag="gemm")
                nc.tensor.matmul(ps[:], gwt(0, j, 0), xsl[0], start=True, stop=False)
                nc.tensor.matmul(ps[:], gwt(0, j, 1), xsl[1], start=False, stop=False)
                nc.tensor.matmul(ps[:], gwt(1, j, 0), gsl[0], start=False, stop=False)
                nc.tensor.matmul(ps[:], gwt(1, j, 1), gsl[1], start=False, stop=True)
                att = attp.tile([128, NPT], BF16, tag=f"att{j}")
                nc.scalar.activation(att[:], ps[:], mybir.ActivationFunctionType.Relu,
                                     bias=biast[j][:], scale=1.0)
                attsq = sqp.tile([128, NPT], BF16, tag=f"sq{j}")
                nc.vector.tensor_mul(attsq[:], att[:], att[:])
                fj = first and j == 0
                lj = last and j == 1
                nc.tensor.matmul(ps_mean[:], smt(t), att[:],
                                 start=fj, stop=lj, skip_group_check=True)
                nc.tensor.matmul(ps_e[:], sett(j, t), att[:],
                                 start=fj, stop=lj, skip_group_check=True)
                nc.tensor.matmul(ps_q[:], smt(t), attsq[:],
                                 start=fj, stop=lj, skip_group_check=True)

        # ---- phase 2: psi for all 16x512 pixels at once ----
        p1 = psip.tile([NT, NPT], F32, tag="p1")
        nc.scalar.activation(p1[:], ps_mean[:], mybir.ActivationFunctionType.Square)
        v = psip.tile([NT, NPT], F32, tag="v")
        nc.vector.tensor_tensor(v[:], ps_q[:], p1[:], op=mybir.AluOpType.subtract)
        sd = psip.tile([NT, NPT], F32, tag="sd")
        nc.scalar.activation(sd[:], v[:], mybir.ActivationFunctionType.Sqrt,
                             bias=epst[:], scale=1.0)
        r = psip.tile([NT, NPT], F32, tag="r")
        nc.vector.reciprocal(r[:], sd[:])
        z = psip.tile([NT, NPT], F32, tag="z")
        nc.vector.tensor_mul(z[:], ps_e[:], r[:])
        psi = psip.tile([NT, NPT], BF16, tag="psi")
        nc.scalar.activation(psi[:], z[:], mybir.ActivationFunctionType.Sigmoid,
                             bias=s1t[:], scale=1.0)

        # ---- phase 3: broadcast psi, gate x into outres ----
        for t in range(NT):
            b, p0 = t // NTILE_PER_B, (t % NTILE_PER_B) * NPT
            bc = bcastps.tile([128, NPT], F32, tag="bcast")
            nc.tensor.matmul(bc[:], indt(t), psi[:], start=True, stop=True,
                             skip_group_check=True)
            for k in range(2):
                xs = xres[:, off(b, k, p0):off(b, k, p0) + NPT]
                os_ = outres[:, off(b, k, p0):off(b, k, p0) + NPT]
                nc.vector.tensor_mul(os_, xs.bitcast(F32), bc[:])

        # ---- output stores: 1 MB chunks ----
        for b in range(BS):
            for h in range(PIX // CHUNK):
                p0 = h * CHUNK
                for k in range(2):
                    nc.sync.dma_start(
                        out_d[b, 128 * k:128 * (k + 1), p0:p0 + CHUNK],
                        outres[:, off(b, k, p0):off(b, k, p0) + CHUNK])

    nc.compile()
    return nc


_NC = None
LAST_RESULT = None


def kernel(x, g, Wx, Wg, Wpsi, ln_gamma, ln_beta, bxg, bpsi):
    global _NC, LAST_RESULT
    _register_ntff_hook()
    if _NC is None:
        _NC = _build()

    x = np.ascontiguousarray(np.asarray(x, dtype=np.float32))
    g = np.ascontiguousarray(np.asarray(g, dtype=np.float32))
    Wx = np.asarray(Wx, dtype=np.float32)
    Wg = np.asarray(Wg, dtype=np.float32)
    Wpsi = np.asarray(Wpsi, dtype=np.float32)
    ln_gamma = np.asarray(ln_gamma, dtype=np.float32)
    ln_beta = np.asarray(ln_beta, dtype=np.float32)
    bxg = np.asarray(bxg, dtype=np.float32)
    bpsi = np.asarray(bpsi, dtype=np.float32)

    # host-side folds
    Wpg = Wpsi[0] * ln_gamma                      # [C]
    s2 = float(Wpg.sum())
    colE = (Wpg - s2 / C).astype(np.float32)      # [C]
    s1 = float(Wpsi[0] @ ln_beta + bpsi[0])

    gw = np.empty((128, 8 * 128), np.float32)
    for s, Wsrc in enumerate((Wx, Wg)):
        for j in range(2):
            for k in range(2):
                i = (s * 2 + j) * 2 + k
                gw[:, i * 128:(i + 1) * 128] = \
                    Wsrc[128 * j:128 * (j + 1), 128 * k:128 * (k + 1)].T
    sm = np.zeros((128, NT * NT), np.float32)
    se = np.zeros((128, 2 * NT * NT), np.float32)
    for t in range(NT):
        sm[:, t * NT + t] = 1.0 / C
        for j in range(2):
            se[:, (j * NT + t) * NT + t] = colE[128 * j:128 * (j + 1)]
    ind = np.zeros((NT, NT * 128), np.float32)
    for t in range(NT):
        ind[t, t * 128:(t + 1) * 128] = 1.0
    bias2 = np.stack([bxg[:128], bxg[128:]])      # [2, 128]
    scal = np.array([s1, LN_EPS], np.float32)

    bf = ml_dtypes.bfloat16
    xr = x.reshape(B, C, PIX)
    gr = g.reshape(B, C, PIX)
    in_maps = []
    for i in range(N_CORES):
        in_maps.append({
            "x": xr[i * BS:(i + 1) * BS],
            "g": gr[i * BS:(i + 1) * BS],
            "gw": gw,
            "sm": sm.astype(bf),
            "se": se.astype(bf),
            "ind": ind.astype(bf),
            "bxg2": bias2,
            "scal": scal,
        })

    trace = bool(os.environ.get("KERNEL_TRACE"))
    res = run_bass_kernel_spmd(_NC, in_maps, list(range(N_CORES)), trace=trace)
    LAST_RESULT = res
    out = np.concatenate([np.asarray(res.results[i]["out"]) for i in range(N_CORES)],
                         axis=0)
    return out.reshape(B, C, H, W)
